# revision 11
# baseline (speedup 1.0000x reference)
"""Trainium2 Bass/Tile kernel: two chained VALID 3x3 convolutions.

    x  [N,3,256,256] --conv(w1)--> h [N,64,254,254] --conv(w2)--> out [N,128,252,252]

Data-parallel over 8 NeuronCores: batch N=16 -> 2 images per core, conv
weights replicated.  Per core the convs are computed as implicit GEMMs on the
tensor engine.  The kernel is tensor-engine issue-rate bound (the HW activity
monitor duty-cycles the PE between 2.4 GHz and 1.2 GHz column rates under
sustained load), so the design minimizes total matmul *columns*:

  conv1: contraction over C0*3*3=27.  Two image-width halves are packed per
         matmul column with a block-diagonal stationary matrix
         (K=54, M=128), so each column produces h for TWO pixels:
         0.5 columns per output pixel.
  conv2: contraction over C1*9=576 done in 5 matmul passes per 2-row chunk
         (vs 9 naive):
           H  = [A;  B ]: A = h rows, B = h shifted down 1 row
                -> pairs (0,dj)+(1,dj), dj=0..2, K=128      (3 passes)
           H2 = [D;  A2]: D = h shifted left 1 col, A2 = h copy
                -> pair (2,1)+(2,0) at K=128                (1 pass)
                -> single (2,2) via D shifted one more col, K=64 (1 pass)
         B/A2/D are bulk SBUF->SBUF DMAs off the critical path.
         PSUM accumulates the 5 matmuls, scalar engine copies to SBUF
         (bf16), DMA to HBM; host casts the bf16 output back to fp32.

Emission order == tensor-engine execution order: conv1 of strip s+1 and the
im2col DMAs of strip s+2 are interleaved between the conv2 chunks of strip
s (conv1 front-loaded 2-per-iteration) so every producer runs ahead of its
consumer even at the boosted (k=8) tensor cadence.
"""

from contextlib import ExitStack

import ml_dtypes
import numpy as np

import concourse.bass as bass
import concourse.mybir as mybir
import concourse.tile as tile
import concourse.bass_utils as bass_utils
from concourse import bacc

N_CORES = 8
FULL_N = 16
C0, C1, C2 = 3, 64, 128

MODE = "bf16"


def _mm_dt():
    return mybir.dt.bfloat16 if MODE == "bf16" else mybir.dt.float32r


def _np_dt():
    return ml_dtypes.bfloat16 if MODE == "bf16" else np.float32


class Geom:
    def __init__(self, npc, h0, w0, ty):
        self.npc = npc          # images per core
        self.h0, self.w0 = h0, w0
        self.h1, self.w1 = h0 - 2, w0 - 2
        self.h2, self.w2 = h0 - 4, w0 - 4
        self.ty = ty            # conv2 output rows per strip
        assert ty % 2 == 0 and self.h2 % ty == 0
        assert self.w1 % 2 == 0
        self.wh = self.w1 // 2  # conv1 half-width (pixels per packed group)


GEOM = Geom(npc=FULL_N // N_CORES, h0=256, w0=256, ty=42)


def _emit(ctx: ExitStack, tc: tile.TileContext, g: Geom, out, x, w1d, w2p, w2q,
          w2r, mm_dt):
    nc = tc.nc
    f32 = mybir.dt.float32
    Copy = mybir.ActivationFunctionType.Copy
    TY, W1, W2, WH = g.ty, g.w1, g.w2, g.wh

    wpool = ctx.enter_context(tc.tile_pool(name="weights", bufs=1))
    b1pool = ctx.enter_context(tc.tile_pool(name="b1", bufs=2))
    hpool = ctx.enter_context(tc.tile_pool(name="h", bufs=2))
    h2pool = ctx.enter_context(tc.tile_pool(name="h2", bufs=2))
    opool = ctx.enter_context(tc.tile_pool(name="o2", bufs=6))
    ps1 = ctx.enter_context(tc.tile_pool(name="ps1", bufs=4, space="PSUM"))
    ps2 = ctx.enter_context(tc.tile_pool(name="ps2", bufs=4, space="PSUM"))

    w1d_sb = wpool.tile([54, 128], mm_dt)
    nc.sync.dma_start(w1d_sb[:], w1d)
    w2p_sb = wpool.tile([128, 3, C2], mm_dt)
    nc.sync.dma_start(w2p_sb[:], w2p)
    w2q_sb = wpool.tile([128, C2], mm_dt)
    nc.sync.dma_start(w2q_sb[:], w2q)
    w2r_sb = wpool.tile([C1, C2], mm_dt)
    nc.sync.dma_start(w2r_sb[:], w2r)

    def im2col(n, y0):
        """Allocate B1 for a strip; return thunks that emit its 18 DMAs.

        Partition (di*3+dj)*3+c      holds x[c, y0+r+di, dj    : dj+WH]
        Partition 27+(di*3+dj)*3+c   holds x[c, y0+r+di, WH+dj : WH+dj+WH]
        """
        B1 = b1pool.tile([54, TY + 2, WH], mm_dt, tag="b1")

        def dma(t9):
            di, dj = divmod(t9, 3)
            nc.sync.dma_start(
                B1[3 * t9:3 * t9 + 3],
                x[n, :, y0 + di:y0 + di + TY + 2, dj:dj + WH])
            nc.sync.dma_start(
                B1[27 + 3 * t9:27 + 3 * t9 + 3],
                x[n, :, y0 + di:y0 + di + TY + 2, WH + dj:WH + dj + WH])
        return B1, [lambda t9=t9: dma(t9) for t9 in range(9)]

    def conv1_alloc():
        H = hpool.tile([128, TY + 2, W1], mm_dt, tag="h")
        H2 = h2pool.tile([128, TY + 2, W1], mm_dt, tag="h2")
        return H, H2

    def conv1_chunk(B1, H, H2, r):
        """h rows r, r+1 (both width-halves): one K=54/M=128 matmul, then
        A (DVE, 2 half-casts), B (scalar, 2 half-copies from PSUM), and the
        per-chunk A2/D SBUF->SBUF DMAs."""
        P1 = ps1.tile([128, 2, WH], f32, tag="p1")
        nc.tensor.matmul(P1[:], w1d_sb[:], B1[:, r:r + 2, :],
                         start=True, stop=True)
        nc.vector.tensor_copy(H[0:C1, r:r + 2, 0:WH], P1[0:C1])
        nc.vector.tensor_copy(H[0:C1, r:r + 2, WH:W1], P1[C1:128])
        # B: row-shifted copy into 64:128 (B row r' = h row r'+1), from PSUM
        if r == 0:
            nc.scalar.activation(H[C1:128, 0:1, 0:WH], P1[0:C1, 1:2, :], Copy)
            nc.scalar.activation(H[C1:128, 0:1, WH:W1], P1[C1:128, 1:2, :],
                                 Copy)
        else:
            nc.scalar.activation(H[C1:128, r - 1:r + 1, 0:WH], P1[0:C1], Copy)
            nc.scalar.activation(H[C1:128, r - 1:r + 1, WH:W1], P1[C1:128],
                                 Copy)
        if r >= 2:
            # H2 rows needed: 2..TY+1.  A2: plain copy; D: col-shifted copy
            nc.sync.dma_start(H2[C1:128, r:r + 2, :], H[0:C1, r:r + 2, :])
            nc.sync.dma_start(H2[0:C1, r:r + 2, 0:W1 - 1],
                              H[0:C1, r:r + 2, 1:W1])

    def conv2_chunk(n, y0, H, H2, t):
        P2 = ps2.tile([C2, 2, W2], f32, tag="p2")
        for dj in range(3):  # pairs: taps (0,dj) + (1,dj), K=128
            nc.tensor.matmul(P2[:], w2p_sb[:, dj, :],
                             H[:, t:t + 2, dj:dj + W2],
                             start=(dj == 0), stop=False)
        # pair: taps (2,1) [D] + (2,0) [A2], K=128
        nc.tensor.matmul(P2[:], w2q_sb[:],
                         H2[:, t + 2:t + 4, 0:W2],
                         start=False, stop=False)
        # single: tap (2,2) via D shifted one more col, K=64
        nc.tensor.matmul(P2[:], w2r_sb[:],
                         H2[0:C1, t + 2:t + 4, 1:1 + W2],
                         start=False, stop=True)
        O2 = opool.tile([C2, 2, W2], mm_dt, tag="o2")
        # alternate the PSUM->SBUF out-cast between DVE and scalar so that
        # neither falls behind the boosted (k=8) tensor cadence
        if (t // 2) % 2 == 0:
            nc.vector.tensor_copy(O2[:], P2[:])
        else:
            nc.scalar.activation(O2[:], P2[:], Copy)
        nc.sync.dma_start(out[n, :, y0 + t:y0 + t + 2, :], O2[:])

    def conv1_work(B1, H, H2):
        return [lambda r=r: conv1_chunk(B1, H, H2, r)
                for r in range(0, TY + 2, 2)]

    strips = [(n, y0) for n in range(g.npc) for y0 in range(0, g.h2, TY)]
    ns = len(strips)

    # prologue: load strip 0, run conv1(0) as a burst, start loading strip 1
    B1_0, dmas = im2col(*strips[0])
    for t in dmas:
        t()
    cur = conv1_alloc()
    for w in conv1_work(B1_0, *cur):
        w()
    B1s = {}
    if ns > 1:
        B1_1, dmas = im2col(*strips[1])
        for t in dmas:
            t()
        B1s[1] = B1_1

    # steady state: conv2(i) interleaved with conv1(i+1) and im2col(i+2),
    # conv1 front-loaded two chunks per iteration
    for i in range(ns):
        n, y0 = strips[i]
        c1work = []
        nxt = None
        if i + 1 < ns:
            nxt = conv1_alloc()
            c1work = conv1_work(B1s.pop(i + 1), *nxt)
        imwork = []
        if i + 2 < ns:
            B1x, imwork = im2col(*strips[i + 2])
            B1s[i + 2] = B1x
        c2work = [lambda t=t: conv2_chunk(n, y0, *cur, t)
                  for t in range(0, TY, 2)]

        # conv1 chunks: double-up for the first EXTRA iterations so the last
        # chunk (and its copies) lands a couple of iterations before the next
        # strip's conv2 needs it
        EXTRA = len(c1work) - len(c2work) + 2
        ci = 0
        for t in range(len(c2work)):
            per = 2 if t < EXTRA else 1
            for _ in range(per):
                if ci < len(c1work):
                    c1work[ci]()
                    ci += 1
            c2work[t]()
            if t < len(imwork):
                imwork[t]()
        while ci < len(c1work):
            c1work[ci]()
            ci += 1
        cur = nxt


def build(g: Geom = GEOM, mm_dt=None):
    if mm_dt is None:
        mm_dt = _mm_dt()
    nc = bacc.Bacc("TRN2", target_bir_lowering=False, debug=False,
                   num_devices=N_CORES)
    x = nc.dram_tensor("x", [g.npc, C0, g.h0, g.w0], mm_dt,
                       kind="ExternalInput").ap()
    w1d = nc.dram_tensor("w1d", [54, 128], mm_dt, kind="ExternalInput").ap()
    w2p = nc.dram_tensor("w2p", [128, 3, C2], mm_dt, kind="ExternalInput").ap()
    w2q = nc.dram_tensor("w2q", [128, C2], mm_dt, kind="ExternalInput").ap()
    w2r = nc.dram_tensor("w2r", [C1, C2], mm_dt, kind="ExternalInput").ap()
    out = nc.dram_tensor("out", [g.npc, C2, g.h2, g.w2], mm_dt,
                         kind="ExternalOutput").ap()
    with tile.TileContext(nc) as tc:
        with ExitStack() as ctx:
            _emit(ctx, tc, g, out, x, w1d, w2p, w2q, w2r, mm_dt)
    nc.compile()
    return nc


def host_round(a: np.ndarray) -> np.ndarray:
    """Cast fp32 to the matmul storage dtype (bf16 cast, or tf32 rounding)."""
    a = np.ascontiguousarray(a, dtype=np.float32)
    if MODE == "bf16":
        return a.astype(ml_dtypes.bfloat16)
    b = a.view(np.uint32).copy()
    b += 0xFFF + ((b >> 13) & 1)
    b &= np.uint32(0xFFFFE000)
    return b.view(np.float32)


def pack_weights(w1: np.ndarray, w2: np.ndarray):
    """Host-side repack so every device DMA is contiguous.

    w1d: block-diagonal [54, 128]: w1d[p, o] = w1t[p, o]; w1d[27+p, 64+o] =
         w1t[p, o], where w1t[p, o] = w1[o, c, di, dj], p = (di*3+dj)*3 + c
    w2p[k, dj, o]: k<64 -> w2[o, k, 0, dj];  k>=64 -> w2[o, k-64, 1, dj]
    w2q[k, o]:     k<64 -> w2[o, k, 2, 1];   k>=64 -> w2[o, k-64, 2, 0]
    w2r[c, o] = w2[o, c, 2, 2]
    """
    w1 = np.ascontiguousarray(np.asarray(w1), dtype=np.float32)
    w2 = np.ascontiguousarray(np.asarray(w2), dtype=np.float32)
    w1t = w1.transpose(2, 3, 1, 0).reshape(27, C1)
    w1d = np.zeros((54, 128), np.float32)
    w1d[0:27, 0:C1] = w1t
    w1d[27:54, C1:128] = w1t
    w2p = np.empty((128, 3, C2), np.float32)
    w2p[:C1] = w2[:, :, 0, :].transpose(1, 2, 0)
    w2p[C1:] = w2[:, :, 1, :].transpose(1, 2, 0)
    w2q = np.empty((128, C2), np.float32)
    w2q[:C1] = w2[:, :, 2, 1].transpose(1, 0)
    w2q[C1:] = w2[:, :, 2, 0].transpose(1, 0)
    w2r = np.ascontiguousarray(w2[:, :, 2, 2].transpose(1, 0))
    return (host_round(w1d), host_round(w2p), host_round(w2q),
            host_round(w2r))


_NC_CACHE: dict = {}


def _get_nc():
    key = ("main", MODE)
    if key not in _NC_CACHE:
        _NC_CACHE[key] = build()
    return _NC_CACHE[key]


def run(x, w1, w2, trace: bool = False):
    """Shard, run on 8 cores, gather.  Returns (out, BassKernelResults)."""
    x = np.ascontiguousarray(np.asarray(x), dtype=np.float32)
    assert x.shape == (FULL_N, C0, GEOM.h0, GEOM.w0), x.shape
    w1d, w2p, w2q, w2r = pack_weights(w1, w2)
    xs = host_round(x).reshape(N_CORES, GEOM.npc, C0, GEOM.h0, GEOM.w0)
    in_maps = [
        {"x": np.ascontiguousarray(xs[c]), "w1d": w1d, "w2p": w2p,
         "w2q": w2q, "w2r": w2r}
        for c in range(N_CORES)
    ]
    nc = _get_nc()
    res = bass_utils.run_bass_kernel_spmd(
        nc, in_maps, core_ids=list(range(N_CORES)), trace=trace)
    out = np.concatenate([r["out"] for r in res.results], axis=0)
    return out.astype(np.float32), res


def kernel(x, w1, w2):
    out, _ = run(x, w1, w2, trace=False)
    return out


# revision 13
# speedup vs baseline: 1.1445x; 1.1445x over previous
"""Trainium2 Bass/Tile kernel: two chained VALID 3x3 convolutions.

    x  [N,3,256,256] --conv(w1)--> h [N,64,254,254] --conv(w2)--> out [N,128,252,252]

Data-parallel over 8 NeuronCores: batch N=16 -> 2 images per core, conv
weights replicated.  Per core the convs are computed as implicit GEMMs on the
tensor engine.  The kernel is tensor-engine issue-rate bound (the HW activity
monitor duty-cycles the PE between 2.4 GHz and 1.2 GHz column rates under
sustained load), so the design minimizes total matmul *columns*.

conv1 ("slot" layout, 0.5 columns per h pixel):
  One K=54 matmul per EVEN h-row j computes [h_j ; h_{j+1}] stacked across
  the 128 PSUM partitions (block-diagonal stationary matrix: taps of row j
  -> out 0:64, taps of row j+1 -> out 64:128).  A single full-width DVE
  cast writes PSUM -> H[:, j, :], which is exactly the [A; B] layout conv2
  consumes: H[0:64, j] = h_j, H[64:128, j] = h_{j+1}.
  The ODD slots of H and the H2 operands are filled with strided bulk
  SBUF->SBUF DMAs sourced from the even-slot casts:
    H[0:64,  odd j] = h_j     <- H[64:128, j-1]   (B-half of even slot)
    H[64:128, odd j] = h_{j+1} <- H[0:64,  j+1]   (A-half of even slot)

conv2: contraction over C1*9=576 in 5 matmul passes per 2-row chunk:
    H  pairs (0,dj)+(1,dj), dj=0..2, K=128                   (3 passes)
    H2 = [D; A2]: D = h shifted left 1 col, A2 = h copy
         pair (2,1)+(2,0) at K=128                           (1 pass)
         single (2,2) via D shifted one more col, K=64       (1 pass)
  PSUM accumulates the 5 matmuls, scalar engine casts to SBUF (bf16),
  DMA to HBM; host casts the bf16 output back to fp32.

Emission order == tensor-engine execution order: conv1 of strip s+1 and the
im2col DMAs of strip s+2 are interleaved between the conv2 chunks of strip
s so every producer runs ahead of its consumer even at the boosted (k=8)
tensor cadence.
"""

from contextlib import ExitStack

import ml_dtypes
import numpy as np

import concourse.bass as bass
import concourse.mybir as mybir
import concourse.tile as tile
import concourse.bass_utils as bass_utils
from concourse import bacc

N_CORES = 8
FULL_N = 16
C0, C1, C2 = 3, 64, 128

MODE = "bf16"


def _mm_dt():
    return mybir.dt.bfloat16 if MODE == "bf16" else mybir.dt.float32r


def _np_dt():
    return ml_dtypes.bfloat16 if MODE == "bf16" else np.float32


class Geom:
    def __init__(self, npc, h0, w0, ty):
        self.npc = npc          # images per core
        self.h0, self.w0 = h0, w0
        self.h1, self.w1 = h0 - 2, w0 - 2
        self.h2, self.w2 = h0 - 4, w0 - 4
        self.ty = ty            # conv2 output rows per strip
        assert ty % 2 == 0 and self.h2 % ty == 0


GEOM = Geom(npc=FULL_N // N_CORES, h0=256, w0=256, ty=42)


def _piece_edges(ty):
    """Even slot values splitting [0, ty] into ~4 pieces for the bulk
    copies."""
    np_ = 4
    edges = sorted({2 * int(round(i * (ty / 2) / np_)) for i in range(np_ + 1)})
    assert edges[0] == 0 and edges[-1] == ty
    return edges


def _emit(ctx: ExitStack, tc: tile.TileContext, g: Geom, out, x, w1d, w2p, w2q,
          w2r, mm_dt):
    nc = tc.nc
    f32 = mybir.dt.float32
    Copy = mybir.ActivationFunctionType.Copy
    TY, W1, W2 = g.ty, g.w1, g.w2

    wpool = ctx.enter_context(tc.tile_pool(name="weights", bufs=1))
    b1pool = ctx.enter_context(tc.tile_pool(name="b1", bufs=2))
    hpool = ctx.enter_context(tc.tile_pool(name="h", bufs=2))
    h2pool = ctx.enter_context(tc.tile_pool(name="h2", bufs=2))
    opool = ctx.enter_context(tc.tile_pool(name="o2", bufs=6))
    ps1 = ctx.enter_context(tc.tile_pool(name="ps1", bufs=3, space="PSUM"))
    ps2 = ctx.enter_context(tc.tile_pool(name="ps2", bufs=4, space="PSUM"))

    w1d_sb = wpool.tile([54, 128], mm_dt)
    nc.sync.dma_start(w1d_sb[:], w1d)
    w2p_sb = wpool.tile([128, 3, C2], mm_dt)
    nc.sync.dma_start(w2p_sb[:], w2p)
    w2q_sb = wpool.tile([128, C2], mm_dt)
    nc.sync.dma_start(w2q_sb[:], w2q)
    w2r_sb = wpool.tile([C1, C2], mm_dt)
    nc.sync.dma_start(w2r_sb[:], w2r)

    def im2col(n, y0):
        """Allocate B1 for a strip; return thunks that emit its 18 DMAs.

        Partition (di*3+dj)*3+c    holds x[c, y0+rr+di,   dj : dj+W1]
        Partition 27+(di*3+dj)*3+c holds x[c, y0+rr+1+di, dj : dj+W1]
        for slot index rr = 0..TY (moving column rr -> h rows rr, rr+1).
        """
        B1 = b1pool.tile([54, TY + 1, W1], mm_dt, tag="b1")

        def dma(t9):
            di, dj = divmod(t9, 3)
            nc.sync.dma_start(
                B1[3 * t9:3 * t9 + 3],
                x[n, :, y0 + di:y0 + di + TY + 1, dj:dj + W1])
            nc.sync.dma_start(
                B1[27 + 3 * t9:27 + 3 * t9 + 3],
                x[n, :, y0 + 1 + di:y0 + 1 + di + TY + 1, dj:dj + W1])
        return B1, [lambda t9=t9: dma(t9) for t9 in range(9)]

    def conv1_alloc():
        H = hpool.tile([128, TY + 1, W1], mm_dt, tag="h")
        H2 = h2pool.tile([128, TY + 2, W1], mm_dt, tag="h2")
        return H, H2

    def conv1_chunk(B1, H, H2, j):
        """Even slot j: one K=54/M=128 matmul producing [h_j ; h_{j+1}],
        one full-width DVE cast into H[:, j, :]."""
        P1 = ps1.tile([128, W1], f32, tag="p1")
        nc.tensor.matmul(P1[:], w1d_sb[:], B1[:, j, :],
                         start=True, stop=True)
        nc.vector.tensor_copy(H[:, j, :], P1[:])

    def conv1_piece(H, H2, jlo, jhi):
        """Bulk strided copies for the window (jlo, jhi] of even slots
        (emitted after the cast of slot jhi):
          odd H slots jlo+1..jhi-1, H2 even rows max(jlo+2,2)..jhi,
          H2 odd rows jlo+3..jhi+1  (H2 rows clipped to [2, TY+1])."""
        # odd slots of H
        nc.sync.dma_start(H[0:C1, jlo + 1:jhi:2, :],
                          H[C1:128, jlo:jhi - 1:2, :])
        nc.sync.dma_start(H[C1:128, jlo + 1:jhi:2, :],
                          H[0:C1, jlo + 2:jhi + 1:2, :])
        # H2 even rows me0..jhi step 2: A2 (plain) and D (col-shifted),
        # sourced from A-halves of even slots
        me0 = max(jlo + 2, 2)
        if jhi >= me0:
            nc.sync.dma_start(H2[C1:128, me0:jhi + 1:2, :],
                              H[0:C1, me0:jhi + 1:2, :])
            nc.sync.dma_start(H2[0:C1, me0:jhi + 1:2, 0:W1 - 1],
                              H[0:C1, me0:jhi + 1:2, 1:W1])
        # H2 odd rows mo0..jhi+1 step 2, sourced from B-halves of even
        # slots m-1
        mo0 = max(jlo + 3, 3)
        mhi = min(jhi + 1, TY + 1)
        if mhi >= mo0:
            nc.sync.dma_start(H2[C1:128, mo0:mhi + 1:2, :],
                              H[C1:128, mo0 - 1:mhi:2, :])
            nc.sync.dma_start(H2[0:C1, mo0:mhi + 1:2, 0:W1 - 1],
                              H[C1:128, mo0 - 1:mhi:2, 1:W1])

    def conv2_chunk(n, y0, H, H2, t):
        P2 = ps2.tile([C2, 2, W2], f32, tag="p2")
        for dj in range(3):  # pairs: taps (0,dj) + (1,dj), K=128
            nc.tensor.matmul(P2[:], w2p_sb[:, dj, :],
                             H[:, t:t + 2, dj:dj + W2],
                             start=(dj == 0), stop=False)
        # pair: taps (2,1) [D] + (2,0) [A2], K=128
        nc.tensor.matmul(P2[:], w2q_sb[:],
                         H2[:, t + 2:t + 4, 0:W2],
                         start=False, stop=False)
        # single: tap (2,2) via D shifted one more col, K=64
        nc.tensor.matmul(P2[:], w2r_sb[:],
                         H2[0:C1, t + 2:t + 4, 1:1 + W2],
                         start=False, stop=True)
        O2 = opool.tile([C2, 2, W2], mm_dt, tag="o2")
        # PSUM->SBUF out-cast on the scalar (Act) engine; DVE owns the
        # conv1 casts
        nc.scalar.activation(O2[:], P2[:], Copy)
        nc.sync.dma_start(out[n, :, y0 + t:y0 + t + 2, :], O2[:])

    EDGES = _piece_edges(TY)

    def conv1_work(B1, H, H2):
        work = []
        ei = 1
        for j in range(0, TY + 2, 2):
            work.append(lambda j=j: conv1_chunk(B1, H, H2, j))
            if ei < len(EDGES) and j == EDGES[ei]:
                jlo, jhi = EDGES[ei - 1], EDGES[ei]
                work.append(lambda jlo=jlo, jhi=jhi:
                            conv1_piece(H, H2, jlo, jhi))
                ei += 1
        return work

    strips = [(n, y0) for n in range(g.npc) for y0 in range(0, g.h2, TY)]
    ns = len(strips)

    # prologue: load strip 0, run conv1(0) as a burst, start loading strip 1
    B1_0, dmas = im2col(*strips[0])
    for t in dmas:
        t()
    cur = conv1_alloc()
    for w in conv1_work(B1_0, *cur):
        w()
    B1s = {}
    if ns > 1:
        B1_1, dmas = im2col(*strips[1])
        for t in dmas:
            t()
        B1s[1] = B1_1

    # steady state: conv2(i) interleaved with conv1(i+1) and im2col(i+2),
    # conv1 front-loaded so its last copies land before the next strip's
    # conv2 needs them
    for i in range(ns):
        n, y0 = strips[i]
        c1work = []
        nxt = None
        if i + 1 < ns:
            nxt = conv1_alloc()
            c1work = conv1_work(B1s.pop(i + 1), *nxt)
        imwork = []
        if i + 2 < ns:
            B1x, imwork = im2col(*strips[i + 2])
            B1s[i + 2] = B1x
        c2work = [lambda t=t: conv2_chunk(n, y0, *cur, t)
                  for t in range(0, TY, 2)]

        EXTRA = max(len(c1work) - len(c2work) + 2, 0)
        ci = 0
        for t in range(len(c2work)):
            per = 2 if t < EXTRA else 1
            for _ in range(per):
                if ci < len(c1work):
                    c1work[ci]()
                    ci += 1
            c2work[t]()
            if t < len(imwork):
                imwork[t]()
        while ci < len(c1work):
            c1work[ci]()
            ci += 1
        cur = nxt


def build(g: Geom = GEOM, mm_dt=None):
    if mm_dt is None:
        mm_dt = _mm_dt()
    nc = bacc.Bacc("TRN2", target_bir_lowering=False, debug=False,
                   num_devices=N_CORES)
    x = nc.dram_tensor("x", [g.npc, C0, g.h0, g.w0], mm_dt,
                       kind="ExternalInput").ap()
    w1d = nc.dram_tensor("w1d", [54, 128], mm_dt, kind="ExternalInput").ap()
    w2p = nc.dram_tensor("w2p", [128, 3, C2], mm_dt, kind="ExternalInput").ap()
    w2q = nc.dram_tensor("w2q", [128, C2], mm_dt, kind="ExternalInput").ap()
    w2r = nc.dram_tensor("w2r", [C1, C2], mm_dt, kind="ExternalInput").ap()
    out = nc.dram_tensor("out", [g.npc, C2, g.h2, g.w2], mm_dt,
                         kind="ExternalOutput").ap()
    with tile.TileContext(nc) as tc:
        with ExitStack() as ctx:
            _emit(ctx, tc, g, out, x, w1d, w2p, w2q, w2r, mm_dt)
    nc.compile()
    return nc


def host_round(a: np.ndarray) -> np.ndarray:
    """Cast fp32 to the matmul storage dtype (bf16 cast, or tf32 rounding)."""
    a = np.ascontiguousarray(a, dtype=np.float32)
    if MODE == "bf16":
        return a.astype(ml_dtypes.bfloat16)
    b = a.view(np.uint32).copy()
    b += 0xFFF + ((b >> 13) & 1)
    b &= np.uint32(0xFFFFE000)
    return b.view(np.float32)


def pack_weights(w1: np.ndarray, w2: np.ndarray):
    """Host-side repack so every device DMA is contiguous.

    w1d: block-diagonal [54, 128]: w1d[p, o] = w1t[p, o]; w1d[27+p, 64+o] =
         w1t[p, o], where w1t[p, o] = w1[o, c, di, dj], p = (di*3+dj)*3 + c
    w2p[k, dj, o]: k<64 -> w2[o, k, 0, dj];  k>=64 -> w2[o, k-64, 1, dj]
    w2q[k, o]:     k<64 -> w2[o, k, 2, 1];   k>=64 -> w2[o, k-64, 2, 0]
    w2r[c, o] = w2[o, c, 2, 2]
    """
    w1 = np.ascontiguousarray(np.asarray(w1), dtype=np.float32)
    w2 = np.ascontiguousarray(np.asarray(w2), dtype=np.float32)
    w1t = w1.transpose(2, 3, 1, 0).reshape(27, C1)
    w1d = np.zeros((54, 128), np.float32)
    w1d[0:27, 0:C1] = w1t
    w1d[27:54, C1:128] = w1t
    w2p = np.empty((128, 3, C2), np.float32)
    w2p[:C1] = w2[:, :, 0, :].transpose(1, 2, 0)
    w2p[C1:] = w2[:, :, 1, :].transpose(1, 2, 0)
    w2q = np.empty((128, C2), np.float32)
    w2q[:C1] = w2[:, :, 2, 1].transpose(1, 0)
    w2q[C1:] = w2[:, :, 2, 0].transpose(1, 0)
    w2r = np.ascontiguousarray(w2[:, :, 2, 2].transpose(1, 0))
    return (host_round(w1d), host_round(w2p), host_round(w2q),
            host_round(w2r))


_NC_CACHE: dict = {}


def _get_nc():
    key = ("main", MODE)
    if key not in _NC_CACHE:
        _NC_CACHE[key] = build()
    return _NC_CACHE[key]


def run(x, w1, w2, trace: bool = False):
    """Shard, run on 8 cores, gather.  Returns (out, BassKernelResults)."""
    x = np.ascontiguousarray(np.asarray(x), dtype=np.float32)
    assert x.shape == (FULL_N, C0, GEOM.h0, GEOM.w0), x.shape
    w1d, w2p, w2q, w2r = pack_weights(w1, w2)
    xs = host_round(x).reshape(N_CORES, GEOM.npc, C0, GEOM.h0, GEOM.w0)
    in_maps = [
        {"x": np.ascontiguousarray(xs[c]), "w1d": w1d, "w2p": w2p,
         "w2q": w2q, "w2r": w2r}
        for c in range(N_CORES)
    ]
    nc = _get_nc()
    res = bass_utils.run_bass_kernel_spmd(
        nc, in_maps, core_ids=list(range(N_CORES)), trace=trace)
    out = np.concatenate([r["out"] for r in res.results], axis=0)
    return out.astype(np.float32), res


def kernel(x, w1, w2):
    out, _ = run(x, w1, w2, trace=False)
    return out


# revision 14
# speedup vs baseline: 1.2708x; 1.1104x over previous
"""Trainium2 Bass/Tile kernel: two chained VALID 3x3 convolutions.

    x  [N,3,256,256] --conv(w1)--> h [N,64,254,254] --conv(w2)--> out [N,128,252,252]

Data-parallel over 8 NeuronCores: batch N=16 -> 2 images per core, conv
weights replicated.  Per core the convs are computed as implicit GEMMs on the
tensor engine.  The kernel is tensor-engine issue-rate bound (the HW activity
monitor duty-cycles the PE between 2.4 GHz and 1.2 GHz column rates under
sustained load), so the design minimizes total matmul *columns*.

conv1 ("slot" layout, 0.5 columns per h pixel):
  One K=54 matmul per EVEN h-row j computes [h_j ; h_{j+1}] stacked across
  the 128 PSUM partitions (block-diagonal stationary matrix: taps of row j
  -> out 0:64, taps of row j+1 -> out 64:128).  A single full-width DVE
  cast writes PSUM -> H[:, j, :], which is exactly the [A; B] layout conv2
  consumes: H[0:64, j] = h_j, H[64:128, j] = h_{j+1}.
  The ODD slots of H and the H2 operands are filled with strided bulk
  SBUF->SBUF DMAs sourced from the even-slot casts:
    H[0:64,  odd j] = h_j     <- H[64:128, j-1]   (B-half of even slot)
    H[64:128, odd j] = h_{j+1} <- H[0:64,  j+1]   (A-half of even slot)

conv2: contraction over C1*9=576 in 5 matmul passes per 2-row chunk:
    H  pairs (0,dj)+(1,dj), dj=0..2, K=128                   (3 passes)
    H2 = [D; A2]: D = h shifted left 1 col, A2 = h copy
         pair (2,1)+(2,0) at K=128                           (1 pass)
         single (2,2) via D shifted one more col, K=64       (1 pass)
  PSUM accumulates the 5 matmuls, scalar engine casts to SBUF (bf16),
  DMA to HBM; host casts the bf16 output back to fp32.

Emission order == tensor-engine execution order: conv1 of strip s+1 and the
im2col DMAs of strip s+2 are interleaved between the conv2 chunks of strip
s so every producer runs ahead of its consumer even at the boosted (k=8)
tensor cadence.
"""

from contextlib import ExitStack

import ml_dtypes
import numpy as np

import concourse.bass as bass
import concourse.mybir as mybir
import concourse.tile as tile
import concourse.bass_utils as bass_utils
from concourse import bacc

N_CORES = 8
FULL_N = 16
C0, C1, C2 = 3, 64, 128

MODE = "bf16"


def _mm_dt():
    return mybir.dt.bfloat16 if MODE == "bf16" else mybir.dt.float32r


def _np_dt():
    return ml_dtypes.bfloat16 if MODE == "bf16" else np.float32


class Geom:
    def __init__(self, npc, h0, w0, ty):
        self.npc = npc          # images per core
        self.h0, self.w0 = h0, w0
        self.h1, self.w1 = h0 - 2, w0 - 2
        self.h2, self.w2 = h0 - 4, w0 - 4
        self.ty = ty            # conv2 output rows per strip
        assert ty % 2 == 0 and self.h2 % ty == 0


GEOM = Geom(npc=FULL_N // N_CORES, h0=256, w0=256, ty=42)


def _piece_edges(ty):
    """Even slot values splitting [0, ty] into ~4 pieces for the bulk
    copies."""
    np_ = 4
    edges = sorted({2 * int(round(i * (ty / 2) / np_)) for i in range(np_ + 1)})
    assert edges[0] == 0 and edges[-1] == ty
    return edges


def _emit(ctx: ExitStack, tc: tile.TileContext, g: Geom, out, x, w1d, w2p, w2q,
          w2r, mm_dt):
    nc = tc.nc
    f32 = mybir.dt.float32
    Copy = mybir.ActivationFunctionType.Copy
    TY, W1, W2 = g.ty, g.w1, g.w2

    wpool = ctx.enter_context(tc.tile_pool(name="weights", bufs=1))
    b1pool = ctx.enter_context(tc.tile_pool(name="b1", bufs=2))
    hpool = ctx.enter_context(tc.tile_pool(name="h", bufs=2))
    h2pool = ctx.enter_context(tc.tile_pool(name="h2", bufs=2))
    opool = ctx.enter_context(tc.tile_pool(name="o2", bufs=6))
    ps1 = ctx.enter_context(tc.tile_pool(name="ps1", bufs=3, space="PSUM"))
    ps2 = ctx.enter_context(tc.tile_pool(name="ps2", bufs=4, space="PSUM"))

    w1d_sb = wpool.tile([54, 128], mm_dt)
    nc.sync.dma_start(w1d_sb[:], w1d)
    w2p_sb = wpool.tile([128, 3, C2], mm_dt)
    nc.sync.dma_start(w2p_sb[:], w2p)
    w2q_sb = wpool.tile([128, C2], mm_dt)
    nc.sync.dma_start(w2q_sb[:], w2q)
    w2r_sb = wpool.tile([C1, C2], mm_dt)
    nc.sync.dma_start(w2r_sb[:], w2r)

    def im2col(n, y0):
        """Allocate B1 for a strip; return thunks that emit its 18 DMAs.

        Partition (di*3+dj)*3+c    holds x[c, y0+rr+di,   dj : dj+W1]
        Partition 27+(di*3+dj)*3+c holds x[c, y0+rr+1+di, dj : dj+W1]
        for slot index rr = 0..TY (moving column rr -> h rows rr, rr+1).
        """
        B1 = b1pool.tile([54, TY + 1, W1], mm_dt, tag="b1")

        def dma(t9):
            di, dj = divmod(t9, 3)
            nc.sync.dma_start(
                B1[3 * t9:3 * t9 + 3],
                x[n, :, y0 + di:y0 + di + TY + 1, dj:dj + W1])
            nc.sync.dma_start(
                B1[27 + 3 * t9:27 + 3 * t9 + 3],
                x[n, :, y0 + 1 + di:y0 + 1 + di + TY + 1, dj:dj + W1])
        return B1, [lambda t9=t9: dma(t9) for t9 in range(9)]

    def conv1_alloc():
        H = hpool.tile([128, TY + 1, W1], mm_dt, tag="h")
        H2 = h2pool.tile([128, TY + 2, W1], mm_dt, tag="h2")
        return H, H2

    def conv1_chunk(B1, H, H2, j):
        """Even slot j: one K=54/M=128 matmul producing [h_j ; h_{j+1}],
        one full-width DVE cast into H[:, j, :].  Then fill the ODD slot
        j-1 = [h_{j-1}; h_j] with two SBUF half-copies (bf16, 2x mode):
        DVE copies h_{j-1} from the B-half of slot j-2, scalar copies h_j
        from the A-half of slot j."""
        P1 = ps1.tile([128, W1], f32, tag="p1")
        nc.tensor.matmul(P1[:], w1d_sb[:], B1[:, j, :],
                         start=True, stop=True)
        nc.vector.tensor_copy(H[:, j, :], P1[:])
        if j >= 2:
            nc.vector.tensor_copy(H[0:C1, j - 1, :], H[C1:128, j - 2, :])
            nc.scalar.activation(H[C1:128, j - 1, :], H[0:C1, j, :], Copy)

    def conv1_piece(H, H2, jlo, jhi):
        """A2/D bulk copies for h rows (jlo, jhi], clipped to [2, TY]
        (emitted after slot jhi's cast and odd-fill, when H[0:C1, m] holds
        h_m for every m <= jhi).  A2 rows are contiguous (one descriptor
        per partition); D is col-shifted (per-row descriptors)."""
        a, b = max(jlo + 1, 2), jhi
        if b >= a:
            nc.sync.dma_start(H2[C1:128, a:b + 1, :], H[0:C1, a:b + 1, :])
            nc.sync.dma_start(H2[0:C1, a:b + 1, 0:W1 - 1],
                              H[0:C1, a:b + 1, 1:W1])
        if jhi == TY:
            # final row TY+1 = B-half of slot TY
            nc.sync.dma_start(H2[C1:128, TY + 1, :], H[C1:128, TY, :])
            nc.sync.dma_start(H2[0:C1, TY + 1, 0:W1 - 1],
                              H[C1:128, TY, 1:W1])

    def conv2_chunk(n, y0, H, H2, t):
        P2 = ps2.tile([C2, 2, W2], f32, tag="p2")
        for dj in range(3):  # pairs: taps (0,dj) + (1,dj), K=128
            nc.tensor.matmul(P2[:], w2p_sb[:, dj, :],
                             H[:, t:t + 2, dj:dj + W2],
                             start=(dj == 0), stop=False)
        # pair: taps (2,1) [D] + (2,0) [A2], K=128
        nc.tensor.matmul(P2[:], w2q_sb[:],
                         H2[:, t + 2:t + 4, 0:W2],
                         start=False, stop=False)
        # single: tap (2,2) via D shifted one more col, K=64
        nc.tensor.matmul(P2[:], w2r_sb[:],
                         H2[0:C1, t + 2:t + 4, 1:1 + W2],
                         start=False, stop=True)
        O2 = opool.tile([C2, 2, W2], mm_dt, tag="o2")
        # PSUM->SBUF out-cast on the scalar (Act) engine; DVE owns the
        # conv1 casts
        nc.scalar.activation(O2[:], P2[:], Copy)
        nc.sync.dma_start(out[n, :, y0 + t:y0 + t + 2, :], O2[:])

    EDGES = _piece_edges(TY)

    def conv1_work(B1, H, H2):
        work = []
        ei = 1
        for j in range(0, TY + 2, 2):
            work.append(lambda j=j: conv1_chunk(B1, H, H2, j))
            if ei < len(EDGES) and j == EDGES[ei]:
                jlo, jhi = EDGES[ei - 1], EDGES[ei]
                work.append(lambda jlo=jlo, jhi=jhi:
                            conv1_piece(H, H2, jlo, jhi))
                ei += 1
        return work

    strips = [(n, y0) for n in range(g.npc) for y0 in range(0, g.h2, TY)]
    ns = len(strips)

    # prologue: load strip 0, run conv1(0) as a burst, start loading strip 1
    B1_0, dmas = im2col(*strips[0])
    for t in dmas:
        t()
    cur = conv1_alloc()
    for w in conv1_work(B1_0, *cur):
        w()
    B1s = {}
    if ns > 1:
        B1_1, dmas = im2col(*strips[1])
        for t in dmas:
            t()
        B1s[1] = B1_1

    # steady state: conv2(i) interleaved with conv1(i+1) and im2col(i+2),
    # conv1 front-loaded so its last copies land before the next strip's
    # conv2 needs them
    for i in range(ns):
        n, y0 = strips[i]
        c1work = []
        nxt = None
        if i + 1 < ns:
            nxt = conv1_alloc()
            c1work = conv1_work(B1s.pop(i + 1), *nxt)
        imwork = []
        if i + 2 < ns:
            B1x, imwork = im2col(*strips[i + 2])
            B1s[i + 2] = B1x
        c2work = [lambda t=t: conv2_chunk(n, y0, *cur, t)
                  for t in range(0, TY, 2)]

        EXTRA = max(len(c1work) - len(c2work) + 2, 0)
        ci = 0
        for t in range(len(c2work)):
            per = 2 if t < EXTRA else 1
            for _ in range(per):
                if ci < len(c1work):
                    c1work[ci]()
                    ci += 1
            c2work[t]()
            if t < len(imwork):
                imwork[t]()
        while ci < len(c1work):
            c1work[ci]()
            ci += 1
        cur = nxt


def build(g: Geom = GEOM, mm_dt=None):
    if mm_dt is None:
        mm_dt = _mm_dt()
    nc = bacc.Bacc("TRN2", target_bir_lowering=False, debug=False,
                   num_devices=N_CORES)
    x = nc.dram_tensor("x", [g.npc, C0, g.h0, g.w0], mm_dt,
                       kind="ExternalInput").ap()
    w1d = nc.dram_tensor("w1d", [54, 128], mm_dt, kind="ExternalInput").ap()
    w2p = nc.dram_tensor("w2p", [128, 3, C2], mm_dt, kind="ExternalInput").ap()
    w2q = nc.dram_tensor("w2q", [128, C2], mm_dt, kind="ExternalInput").ap()
    w2r = nc.dram_tensor("w2r", [C1, C2], mm_dt, kind="ExternalInput").ap()
    out = nc.dram_tensor("out", [g.npc, C2, g.h2, g.w2], mm_dt,
                         kind="ExternalOutput").ap()
    with tile.TileContext(nc) as tc:
        with ExitStack() as ctx:
            _emit(ctx, tc, g, out, x, w1d, w2p, w2q, w2r, mm_dt)
    nc.compile()
    return nc


def host_round(a: np.ndarray) -> np.ndarray:
    """Cast fp32 to the matmul storage dtype (bf16 cast, or tf32 rounding)."""
    a = np.ascontiguousarray(a, dtype=np.float32)
    if MODE == "bf16":
        return a.astype(ml_dtypes.bfloat16)
    b = a.view(np.uint32).copy()
    b += 0xFFF + ((b >> 13) & 1)
    b &= np.uint32(0xFFFFE000)
    return b.view(np.float32)


def pack_weights(w1: np.ndarray, w2: np.ndarray):
    """Host-side repack so every device DMA is contiguous.

    w1d: block-diagonal [54, 128]: w1d[p, o] = w1t[p, o]; w1d[27+p, 64+o] =
         w1t[p, o], where w1t[p, o] = w1[o, c, di, dj], p = (di*3+dj)*3 + c
    w2p[k, dj, o]: k<64 -> w2[o, k, 0, dj];  k>=64 -> w2[o, k-64, 1, dj]
    w2q[k, o]:     k<64 -> w2[o, k, 2, 1];   k>=64 -> w2[o, k-64, 2, 0]
    w2r[c, o] = w2[o, c, 2, 2]
    """
    w1 = np.ascontiguousarray(np.asarray(w1), dtype=np.float32)
    w2 = np.ascontiguousarray(np.asarray(w2), dtype=np.float32)
    w1t = w1.transpose(2, 3, 1, 0).reshape(27, C1)
    w1d = np.zeros((54, 128), np.float32)
    w1d[0:27, 0:C1] = w1t
    w1d[27:54, C1:128] = w1t
    w2p = np.empty((128, 3, C2), np.float32)
    w2p[:C1] = w2[:, :, 0, :].transpose(1, 2, 0)
    w2p[C1:] = w2[:, :, 1, :].transpose(1, 2, 0)
    w2q = np.empty((128, C2), np.float32)
    w2q[:C1] = w2[:, :, 2, 1].transpose(1, 0)
    w2q[C1:] = w2[:, :, 2, 0].transpose(1, 0)
    w2r = np.ascontiguousarray(w2[:, :, 2, 2].transpose(1, 0))
    return (host_round(w1d), host_round(w2p), host_round(w2q),
            host_round(w2r))


_NC_CACHE: dict = {}


def _get_nc():
    key = ("main", MODE)
    if key not in _NC_CACHE:
        _NC_CACHE[key] = build()
    return _NC_CACHE[key]


def run(x, w1, w2, trace: bool = False):
    """Shard, run on 8 cores, gather.  Returns (out, BassKernelResults)."""
    x = np.ascontiguousarray(np.asarray(x), dtype=np.float32)
    assert x.shape == (FULL_N, C0, GEOM.h0, GEOM.w0), x.shape
    w1d, w2p, w2q, w2r = pack_weights(w1, w2)
    xs = host_round(x).reshape(N_CORES, GEOM.npc, C0, GEOM.h0, GEOM.w0)
    in_maps = [
        {"x": np.ascontiguousarray(xs[c]), "w1d": w1d, "w2p": w2p,
         "w2q": w2q, "w2r": w2r}
        for c in range(N_CORES)
    ]
    nc = _get_nc()
    res = bass_utils.run_bass_kernel_spmd(
        nc, in_maps, core_ids=list(range(N_CORES)), trace=trace)
    out = np.concatenate([r["out"] for r in res.results], axis=0)
    return out.astype(np.float32), res


def kernel(x, w1, w2):
    out, _ = run(x, w1, w2, trace=False)
    return out


# revision 15
# speedup vs baseline: 1.5199x; 1.1960x over previous
"""Trainium2 Bass/Tile kernel: two chained VALID 3x3 convolutions.

    x  [N,3,256,256] --conv(w1)--> h [N,64,254,254] --conv(w2)--> out [N,128,252,252]

Data-parallel over 8 NeuronCores: batch N=16 -> 2 images per core, conv
weights replicated.  Per core the convs are computed as implicit GEMMs on the
tensor engine.  The kernel is tensor-engine issue-rate bound (the HW activity
monitor duty-cycles the PE between 2.4 GHz and 1.2 GHz column rates under
sustained load), so the design minimizes total matmul *columns*.

conv1 ("slot" layout, 0.5 columns per h pixel):
  One K=54 matmul per EVEN h-row j computes [h_j ; h_{j+1}] stacked across
  the 128 PSUM partitions (block-diagonal stationary matrix: taps of row j
  -> out 0:64, taps of row j+1 -> out 64:128).  A single full-width DVE
  cast writes PSUM -> H[:, j, :], which is exactly the [A; B] layout conv2
  consumes: H[0:64, j] = h_j, H[64:128, j] = h_{j+1}.
  The ODD slots of H and the H2 operands are filled with strided bulk
  SBUF->SBUF DMAs sourced from the even-slot casts:
    H[0:64,  odd j] = h_j     <- H[64:128, j-1]   (B-half of even slot)
    H[64:128, odd j] = h_{j+1} <- H[0:64,  j+1]   (A-half of even slot)

conv2: contraction over C1*9=576 in 5 matmul passes per 2-row chunk:
    H  pairs (0,dj)+(1,dj), dj=0..2, K=128                   (3 passes)
    H2 = [D; A2]: D = h shifted left 1 col, A2 = h copy
         pair (2,1)+(2,0) at K=128                           (1 pass)
         single (2,2) via D shifted one more col, K=64       (1 pass)
  PSUM accumulates the 5 matmuls, scalar engine casts to SBUF (bf16),
  DMA to HBM; host casts the bf16 output back to fp32.

Emission order == tensor-engine execution order: conv1 of strip s+1 and the
im2col DMAs of strip s+2 are interleaved between the conv2 chunks of strip
s so every producer runs ahead of its consumer even at the boosted (k=8)
tensor cadence.
"""

from contextlib import ExitStack

import ml_dtypes
import numpy as np

import concourse.bass as bass
import concourse.mybir as mybir
import concourse.tile as tile
import concourse.bass_utils as bass_utils
from concourse import bacc

N_CORES = 8
FULL_N = 16
C0, C1, C2 = 3, 64, 128

MODE = "bf16"


def _mm_dt():
    return mybir.dt.bfloat16 if MODE == "bf16" else mybir.dt.float32r


def _np_dt():
    return ml_dtypes.bfloat16 if MODE == "bf16" else np.float32


class Geom:
    def __init__(self, npc, h0, w0, ty):
        self.npc = npc          # images per core
        self.h0, self.w0 = h0, w0
        self.h1, self.w1 = h0 - 2, w0 - 2
        self.h2, self.w2 = h0 - 4, w0 - 4
        self.ty = ty            # conv2 output rows per strip
        assert ty % 2 == 0 and self.h2 % ty == 0


GEOM = Geom(npc=FULL_N // N_CORES, h0=256, w0=256, ty=42)


def _piece_edges(ty):
    """Even slot values splitting [0, ty] into ~4 pieces for the bulk
    copies."""
    np_ = 4
    edges = sorted({2 * int(round(i * (ty / 2) / np_)) for i in range(np_ + 1)})
    assert edges[0] == 0 and edges[-1] == ty
    return edges


def _emit(ctx: ExitStack, tc: tile.TileContext, g: Geom, out, x, w1d, w2p, w2q,
          w2r, mm_dt):
    nc = tc.nc
    f32 = mybir.dt.float32
    Copy = mybir.ActivationFunctionType.Copy
    TY, W1, W2 = g.ty, g.w1, g.w2

    wpool = ctx.enter_context(tc.tile_pool(name="weights", bufs=1))
    b1pool = ctx.enter_context(tc.tile_pool(name="b1", bufs=2))
    hpool = ctx.enter_context(tc.tile_pool(name="h", bufs=2))
    h2pool = ctx.enter_context(tc.tile_pool(name="h2", bufs=2))
    opool = ctx.enter_context(tc.tile_pool(name="o2", bufs=6))
    ps1 = ctx.enter_context(tc.tile_pool(name="ps1", bufs=3, space="PSUM"))
    ps2 = ctx.enter_context(tc.tile_pool(name="ps2", bufs=4, space="PSUM"))

    w1d_sb = wpool.tile([54, 128], mm_dt)
    nc.sync.dma_start(w1d_sb[:], w1d)
    w2p_sb = wpool.tile([128, 3, C2], mm_dt)
    nc.sync.dma_start(w2p_sb[:], w2p)
    w2q_sb = wpool.tile([128, C2], mm_dt)
    nc.sync.dma_start(w2q_sb[:], w2q)
    w2r_sb = wpool.tile([C1, C2], mm_dt)
    nc.sync.dma_start(w2r_sb[:], w2r)

    def im2col(n, y0):
        """Allocate B1 for a strip; return thunks that emit its 18 DMAs.

        Partition (di*3+dj)*3+c    holds x[c, y0+rr+di,   dj : dj+W1]
        Partition 27+(di*3+dj)*3+c holds x[c, y0+rr+1+di, dj : dj+W1]
        for slot index rr = 0..TY (moving column rr -> h rows rr, rr+1).
        """
        B1 = b1pool.tile([54, TY + 1, W1], mm_dt, tag="b1")

        def dma(t9):
            di, dj = divmod(t9, 3)
            nc.sync.dma_start(
                B1[3 * t9:3 * t9 + 3],
                x[n, :, y0 + di:y0 + di + TY + 1, dj:dj + W1])
            nc.sync.dma_start(
                B1[27 + 3 * t9:27 + 3 * t9 + 3],
                x[n, :, y0 + 1 + di:y0 + 1 + di + TY + 1, dj:dj + W1])
        return B1, [lambda t9=t9: dma(t9) for t9 in range(9)]

    def conv1_alloc():
        H = hpool.tile([128, TY + 1, W1], mm_dt, tag="h")
        H2 = h2pool.tile([128, TY + 2, W1], mm_dt, tag="h2")
        return H, H2

    def conv1_chunk(B1, H, H2, j):
        """Even slot j: one K=54/M=128 matmul producing [h_j ; h_{j+1}],
        one full-width DVE cast into H[:, j, :].  Then fill the ODD slot
        j-1 = [h_{j-1}; h_j] with two SBUF half-copies (bf16, 2x mode):
        DVE copies h_{j-1} from the B-half of slot j-2, scalar copies h_j
        from the A-half of slot j."""
        P1 = ps1.tile([128, W1], f32, tag="p1")
        nc.tensor.matmul(P1[:], w1d_sb[:], B1[:, j, :],
                         start=True, stop=True)
        nc.vector.tensor_copy(H[:, j, :], P1[:])
        if j >= 2:
            nc.vector.tensor_copy(H[0:C1, j - 1, :], H[C1:128, j - 2, :])
            nc.scalar.activation(H[C1:128, j - 1, :], H[0:C1, j, :], Copy)

    def conv1_piece(H, H2, jlo, jhi, pidx):
        """A2/D bulk copies for h rows (jlo, jhi], clipped to [2, TY]
        (emitted after slot jhi's cast and odd-fill, when H[0:C1, m] holds
        h_m for every m <= jhi).  A2 rows are contiguous SBUF->SBUF DMA
        (one descriptor per partition); D (col-shifted) runs on the
        compute engines (bf16 2x) -- mostly DVE, last piece on scalar --
        so the small-packet copies stay off the DMA queues."""
        a, b = max(jlo + 1, 2), jhi
        if b >= a:
            nc.sync.dma_start(H2[C1:128, a:b + 1, :], H[0:C1, a:b + 1, :])
            if pidx < 3:
                nc.vector.tensor_copy(H2[0:C1, a:b + 1, 0:W1 - 1],
                                      H[0:C1, a:b + 1, 1:W1])
            else:
                nc.scalar.activation(H2[0:C1, a:b + 1, 0:W1 - 1],
                                     H[0:C1, a:b + 1, 1:W1], Copy)
        if jhi == TY:
            # final row TY+1 = B-half of slot TY
            nc.sync.dma_start(H2[C1:128, TY + 1, :], H[C1:128, TY, :])
            nc.scalar.activation(H2[0:C1, TY + 1, 0:W1 - 1],
                                 H[C1:128, TY, 1:W1], Copy)

    def conv2_chunk(n, y0, H, H2, t):
        P2 = ps2.tile([C2, 2, W2], f32, tag="p2")
        for dj in range(3):  # pairs: taps (0,dj) + (1,dj), K=128
            nc.tensor.matmul(P2[:], w2p_sb[:, dj, :],
                             H[:, t:t + 2, dj:dj + W2],
                             start=(dj == 0), stop=False)
        # pair: taps (2,1) [D] + (2,0) [A2], K=128
        nc.tensor.matmul(P2[:], w2q_sb[:],
                         H2[:, t + 2:t + 4, 0:W2],
                         start=False, stop=False)
        # single: tap (2,2) via D shifted one more col, K=64
        nc.tensor.matmul(P2[:], w2r_sb[:],
                         H2[0:C1, t + 2:t + 4, 1:1 + W2],
                         start=False, stop=True)
        O2 = opool.tile([C2, 2, W2], mm_dt, tag="o2")
        # PSUM->SBUF out-cast on the scalar (Act) engine; DVE owns the
        # conv1 casts
        nc.scalar.activation(O2[:], P2[:], Copy)
        nc.sync.dma_start(out[n, :, y0 + t:y0 + t + 2, :], O2[:])

    EDGES = _piece_edges(TY)

    def conv1_work(B1, H, H2):
        work = []
        ei = 1
        for j in range(0, TY + 2, 2):
            work.append(lambda j=j: conv1_chunk(B1, H, H2, j))
            if ei < len(EDGES) and j == EDGES[ei]:
                jlo, jhi = EDGES[ei - 1], EDGES[ei]
                work.append(lambda jlo=jlo, jhi=jhi, pidx=ei - 1:
                            conv1_piece(H, H2, jlo, jhi, pidx))
                ei += 1
        return work

    strips = [(n, y0) for n in range(g.npc) for y0 in range(0, g.h2, TY)]
    ns = len(strips)

    # prologue: load strip 0, run conv1(0) as a burst, start loading strip 1
    B1_0, dmas = im2col(*strips[0])
    for t in dmas:
        t()
    cur = conv1_alloc()
    for w in conv1_work(B1_0, *cur):
        w()
    B1s = {}
    if ns > 1:
        B1_1, dmas = im2col(*strips[1])
        for t in dmas:
            t()
        B1s[1] = B1_1

    # steady state: conv2(i) interleaved with conv1(i+1) and im2col(i+2),
    # conv1 front-loaded so its last copies land before the next strip's
    # conv2 needs them
    for i in range(ns):
        n, y0 = strips[i]
        c1work = []
        nxt = None
        if i + 1 < ns:
            nxt = conv1_alloc()
            c1work = conv1_work(B1s.pop(i + 1), *nxt)
        imwork = []
        if i + 2 < ns:
            B1x, imwork = im2col(*strips[i + 2])
            B1s[i + 2] = B1x
        c2work = [lambda t=t: conv2_chunk(n, y0, *cur, t)
                  for t in range(0, TY, 2)]

        EXTRA = max(len(c1work) - len(c2work) + 2, 0)
        ci = 0
        for t in range(len(c2work)):
            per = 2 if t < EXTRA else 1
            for _ in range(per):
                if ci < len(c1work):
                    c1work[ci]()
                    ci += 1
            c2work[t]()
            if t < len(imwork):
                imwork[t]()
        while ci < len(c1work):
            c1work[ci]()
            ci += 1
        cur = nxt


def build(g: Geom = GEOM, mm_dt=None):
    if mm_dt is None:
        mm_dt = _mm_dt()
    nc = bacc.Bacc("TRN2", target_bir_lowering=False, debug=False,
                   num_devices=N_CORES)
    x = nc.dram_tensor("x", [g.npc, C0, g.h0, g.w0], mm_dt,
                       kind="ExternalInput").ap()
    w1d = nc.dram_tensor("w1d", [54, 128], mm_dt, kind="ExternalInput").ap()
    w2p = nc.dram_tensor("w2p", [128, 3, C2], mm_dt, kind="ExternalInput").ap()
    w2q = nc.dram_tensor("w2q", [128, C2], mm_dt, kind="ExternalInput").ap()
    w2r = nc.dram_tensor("w2r", [C1, C2], mm_dt, kind="ExternalInput").ap()
    out = nc.dram_tensor("out", [g.npc, C2, g.h2, g.w2], mm_dt,
                         kind="ExternalOutput").ap()
    with tile.TileContext(nc) as tc:
        with ExitStack() as ctx:
            _emit(ctx, tc, g, out, x, w1d, w2p, w2q, w2r, mm_dt)
    nc.compile()
    return nc


def host_round(a: np.ndarray) -> np.ndarray:
    """Cast fp32 to the matmul storage dtype (bf16 cast, or tf32 rounding)."""
    a = np.ascontiguousarray(a, dtype=np.float32)
    if MODE == "bf16":
        return a.astype(ml_dtypes.bfloat16)
    b = a.view(np.uint32).copy()
    b += 0xFFF + ((b >> 13) & 1)
    b &= np.uint32(0xFFFFE000)
    return b.view(np.float32)


def pack_weights(w1: np.ndarray, w2: np.ndarray):
    """Host-side repack so every device DMA is contiguous.

    w1d: block-diagonal [54, 128]: w1d[p, o] = w1t[p, o]; w1d[27+p, 64+o] =
         w1t[p, o], where w1t[p, o] = w1[o, c, di, dj], p = (di*3+dj)*3 + c
    w2p[k, dj, o]: k<64 -> w2[o, k, 0, dj];  k>=64 -> w2[o, k-64, 1, dj]
    w2q[k, o]:     k<64 -> w2[o, k, 2, 1];   k>=64 -> w2[o, k-64, 2, 0]
    w2r[c, o] = w2[o, c, 2, 2]
    """
    w1 = np.ascontiguousarray(np.asarray(w1), dtype=np.float32)
    w2 = np.ascontiguousarray(np.asarray(w2), dtype=np.float32)
    w1t = w1.transpose(2, 3, 1, 0).reshape(27, C1)
    w1d = np.zeros((54, 128), np.float32)
    w1d[0:27, 0:C1] = w1t
    w1d[27:54, C1:128] = w1t
    w2p = np.empty((128, 3, C2), np.float32)
    w2p[:C1] = w2[:, :, 0, :].transpose(1, 2, 0)
    w2p[C1:] = w2[:, :, 1, :].transpose(1, 2, 0)
    w2q = np.empty((128, C2), np.float32)
    w2q[:C1] = w2[:, :, 2, 1].transpose(1, 0)
    w2q[C1:] = w2[:, :, 2, 0].transpose(1, 0)
    w2r = np.ascontiguousarray(w2[:, :, 2, 2].transpose(1, 0))
    return (host_round(w1d), host_round(w2p), host_round(w2q),
            host_round(w2r))


_NC_CACHE: dict = {}


def _get_nc():
    key = ("main", MODE)
    if key not in _NC_CACHE:
        _NC_CACHE[key] = build()
    return _NC_CACHE[key]


def run(x, w1, w2, trace: bool = False):
    """Shard, run on 8 cores, gather.  Returns (out, BassKernelResults)."""
    x = np.ascontiguousarray(np.asarray(x), dtype=np.float32)
    assert x.shape == (FULL_N, C0, GEOM.h0, GEOM.w0), x.shape
    w1d, w2p, w2q, w2r = pack_weights(w1, w2)
    xs = host_round(x).reshape(N_CORES, GEOM.npc, C0, GEOM.h0, GEOM.w0)
    in_maps = [
        {"x": np.ascontiguousarray(xs[c]), "w1d": w1d, "w2p": w2p,
         "w2q": w2q, "w2r": w2r}
        for c in range(N_CORES)
    ]
    nc = _get_nc()
    res = bass_utils.run_bass_kernel_spmd(
        nc, in_maps, core_ids=list(range(N_CORES)), trace=trace)
    out = np.concatenate([r["out"] for r in res.results], axis=0)
    return out.astype(np.float32), res


def kernel(x, w1, w2):
    out, _ = run(x, w1, w2, trace=False)
    return out
